# revision 8
# baseline (speedup 1.0000x reference)
"""Multi-head attention (B=2, S=2048, D=1024, H=16) on 8 TRN2 NeuronCores, v2.

Sharding: tensor parallel over heads (2 heads/core). Per core:
  - QKV projection of the full 4096 rows onto its 128 channels.
  - Scores with K=64 contraction per head (tile_position picks the
    64-partition quadrant; no zero padding).
  - Context in [q, ch] orientation: lhsT = exp-scores [keys, q-block],
    rhs = [V | 1] rows -> full 128x128 PE utilization and the ones
    column lands the softmax denominator in the same PSUM tile, so the
    normalize is a per-partition tensor_scalar multiply on DVE.
  - ctx transposed back to [ch, q] on the PE (small), shipped into one of
    FOUR pipelined AllToAll quarters (256KB each), then a row-parallel
    output projection per quarter.

Emission is software-pipelined: scores of chunk i+1 interleave with the
context of chunk i; projection of batch 1 and the phase-2 output
projections ride in the gaps.
"""

import numpy as np

B, S, D, H = 2, 2048, 1024, 16
NCORES = 8
CH = D // NCORES          # 128 channels (2 heads) per core
HD = D // H               # 64
ROWS = B * S              # 4096
RPC = ROWS // NCORES      # 512 output rows per core
KO = D // 128             # 8 contraction chunks of 128
QCH = 512                 # q-chunk (rows) per attention chunk
NCH = S // QCH            # 4 chunks per batch
NKB = S // 128            # 16 key blocks
KBP = NKB // 2            # 8 key-block pairs
NQB = QCH // 128          # 4 q-blocks of 128 per chunk
NQTR = 4                  # AllToAll quarters
RH = 128                  # rows per core per quarter
SCALE = 1.0 / 32.0        # 1/sqrt(D)
WS = 32.0                 # fp8 weight pre-scale for Wq/Wk
SCALE8 = SCALE / (WS * WS)  # exp scale when q,k carry WS each

_CACHE = {}


def _build():
    import concourse.mybir as mybir
    import concourse.tile as tile
    from concourse import bacc
    from concourse.masks import make_identity

    BF16 = mybir.dt.bfloat16
    F32 = mybir.dt.float32
    AF = mybir.ActivationFunctionType

    F8 = mybir.dt.float8e4
    DR = mybir.MatmulPerfMode.DoubleRow

    nc = bacc.Bacc("TRN2", target_bir_lowering=False, debug=False, num_devices=NCORES)
    xT = nc.dram_tensor("xT", [D, ROWS], BF16, kind="ExternalInput")
    xT8 = nc.dram_tensor("xT8", [D, ROWS], F8, kind="ExternalInput")
    wq = nc.dram_tensor("wq", [128, KO, CH], F8, kind="ExternalInput")
    wk = nc.dram_tensor("wk", [128, KO, CH], F8, kind="ExternalInput")
    wv = nc.dram_tensor("wv", [128, KO, CH], BF16, kind="ExternalInput")
    wo = nc.dram_tensor("wo", [128, KO, D], BF16, kind="ExternalInput")
    out = nc.dram_tensor("out", [RPC, D], F32, kind="ExternalOutput")

    with tile.TileContext(nc) as tc:
        with (
            tc.tile_pool(name="const", bufs=1) as cpool,
            tc.tile_pool(name="xt", bufs=3) as xtp,
            tc.tile_pool(name="qk", bufs=8) as qkp,
            tc.tile_pool(name="vr", bufs=8) as vrp,
            tc.tile_pool(name="e", bufs=32) as ep,
            tc.tile_pool(name="sm", bufs=4) as smp,
            tc.tile_pool(name="cs", bufs=4) as csp,
            tc.tile_pool(name="osb", bufs=2) as osbp,
            tc.tile_pool(name="ctxg", bufs=2) as ctxgp,
            tc.tile_pool(name="ps", bufs=2, space="PSUM") as ps,
            tc.tile_pool(name="dram", bufs=1, space="DRAM") as dram,
        ):
            w_tiles = {}
            for name, t, dt_ in (("wq", wq, F8), ("wk", wk, F8), ("wv", wv, BF16)):
                wt = cpool.tile([128, KO, CH], dt_, tag=name, name=name + "_t")
                w_tiles[name] = (wt, t)
            # wq first, then the first x block, so the first matmul launches
            # as early as the DMA engines allow
            nc.sync.dma_start(w_tiles["wq"][0][:], wq[:])
            ident = cpool.tile([128, 128], BF16, tag="ident")
            make_identity(nc, ident[:])

            a2a_in = [dram.tile([NCORES, CH, RH], BF16, name=f"a2a_in{q}")
                      for q in range(NQTR)]
            a2a_out = [dram.tile([NCORES, CH, RH], BF16, name=f"a2a_out{q}")
                      for q in range(NQTR)]

            xT_r = xT.ap().rearrange("(ko p) n -> p ko n", p=128)
            xT8_r = xT8.ap().rearrange("(ko p) n -> p ko n", p=128)
            w_loaded = {"wq"}

            def ensure_w(name):
                if name not in w_loaded:
                    w_loaded.add(name)
                    nc.sync.dma_start(w_tiles[name][0][:], w_tiles[name][1][:])

            # persistent state, indexed by chunk c = 4*b + k
            kts = {}   # (b, rb) -> kt tile [128ch, 512 keys]
            vrs = {}   # (b, rb) -> vr tile [128 keys, 4, 130]
            qts = {}   # chunk -> qt tile [128ch, 512 q]
            es = {}    # (chunk, h, kbp) -> e tile [128 keys, 2, 512]
            wo_holder = {}

            # ---------------- work items ----------------

            def proj_q(b, rb):
                """x8 DMA + fp8 DoubleRow Q projection; stores qt."""
                x8 = xtp.tile([128, KO, 512], F8, tag="x8", name=f"x8_{b}_{rb}")
                nc.sync.dma_start(x8[:], xT8_r[:, :, (b * NCH + rb) * 512:
                                                (b * NCH + rb + 1) * 512])
                kts[("x8", b, rb)] = x8
                pj = ps.tile([128, 512], F32, tag="big", name=f"qp{b}_{rb}")
                for t in range(KO // 2):
                    nc.tensor.matmul(pj[:],
                                     w_tiles["wq"][0][:, 2 * t:2 * t + 2, :],
                                     x8[:, 2 * t:2 * t + 2, :],
                                     start=(t == 0), stop=(t == KO // 2 - 1),
                                     perf_mode=DR)
                qt = qkp.tile([128, 512], BF16, tag="qt", name=f"qt{b}_{rb}")
                nc.vector.tensor_copy(qt[:], pj[:])
                qts[4 * b + rb] = qt

            def proj_k(b, rb):
                ensure_w("wk")
                x8 = kts[("x8", b, rb)]
                pj = ps.tile([128, 512], F32, tag="big", name=f"kp{b}_{rb}")
                for t in range(KO // 2):
                    nc.tensor.matmul(pj[:],
                                     w_tiles["wk"][0][:, 2 * t:2 * t + 2, :],
                                     x8[:, 2 * t:2 * t + 2, :],
                                     start=(t == 0), stop=(t == KO // 2 - 1),
                                     perf_mode=DR)
                kt = qkp.tile([128, 512], BF16, tag="kt", name=f"kt{b}_{rb}")
                nc.vector.tensor_copy(kt[:], pj[:])
                kts[(b, rb)] = kt

            def proj_v(b, rb):
                """V directly in [keys, ch] orientation; fused ones columns."""
                ensure_w("wv")
                xt = xtp.tile([128, KO, 512], BF16, tag="xt", name=f"xt{b}_{rb}")
                nc.sync.dma_start(xt[:], xT_r[:, :, (b * NCH + rb) * 512:
                                               (b * NCH + rb + 1) * 512])
                vr = vrp.tile([128, 4, 130], BF16, tag="vr", name=f"vr{b}_{rb}")
                nc.vector.memset(vr[:, :, 64:65], 1.0)
                nc.vector.memset(vr[:, :, 129:130], 1.0)
                for kj in range(4):
                    vp = ps.tile([128, 128], F32, tag="big", name=f"vp{b}_{rb}_{kj}")
                    for ko in range(KO):
                        nc.tensor.matmul(
                            vp[:], xt[:, ko, kj * 128:(kj + 1) * 128],
                            w_tiles["wv"][0][:, ko, :],
                            start=(ko == 0), stop=(ko == KO - 1))
                    nc.vector.tensor_copy(vr[:, kj, 0:64], vp[:, 0:64])
                    nc.vector.tensor_copy(vr[:, kj, 65:129], vp[:, 64:128])
                vrs[(b, rb)] = vr

            def score_pair(c, h, kbp):
                """scores for key blocks (2*kbp, 2*kbp+1), head h + exp."""
                b, k = c // NCH, c % NCH
                scs = ps.tile([128, 2, 512], F32, tag="scs",
                              name=f"scs{c}_{h}_{kbp}")
                for t in range(2):
                    kb = 2 * kbp + t
                    krb, kj = kb // 4, kb % 4
                    nc.tensor.matmul(
                        scs[:, t, :],
                        kts[(b, krb)][h * 64:(h + 1) * 64, kj * 128:(kj + 1) * 128],
                        qts[c][h * 64:(h + 1) * 64, :],
                        start=True, stop=True)
                e = ep.tile([128, 2, 512], BF16, tag="e", name=f"e{c}_{h}_{kbp}")
                nc.scalar.activation(e[:], scs[:], AF.Exp, scale=SCALE8)
                es[(c, h, kbp)] = e

            def ctx_group(c, h, qb):
                """context accumulation + normalize for one (head, q-block);
                after h==1 the combined two-head tile is transposed and
                shipped into the AllToAll buffer."""
                b, k = c // NCH, c % NCH
                key = ("ctxp", c, qb)
                if key not in es:
                    es[key] = ps.tile([128, 2, 65], F32, tag="cx",
                                      name=f"ctxp{c}_{qb}")
                    es[("cn2", c, qb)] = smp.tile([128, 128], BF16, tag="cn",
                                                  name=f"cn{c}_{qb}")
                ctxp = es[key]
                cn2 = es[("cn2", c, qb)]
                for kc in range(NKB):
                    kbp, t = kc // 2, kc % 2
                    nc.tensor.matmul(
                        ctxp[:, h, :],
                        es[(c, h, kbp)][:, t, qb * 128:(qb + 1) * 128],
                        vrs[(b, kc // 4)][:, kc % 4, 65 * h:65 * h + 65],
                        start=(kc == 0), stop=(kc == NKB - 1))
                rc = smp.tile([128, 1], F32, tag="rc", name=f"rc{c}_{h}_{qb}")
                nc.vector.reciprocal(rc[:], ctxp[:, h, 64:65])
                nc.vector.tensor_scalar_mul(
                    cn2[:, h * 64:(h + 1) * 64], ctxp[:, h, 0:64], rc[:])
                if h == 1:
                    tp = ps.tile([128, 128], BF16, tag="cx", name=f"tp{c}_{qb}")
                    nc.tensor.transpose(tp[:], cn2[:], ident[:])
                    cs = csp.tile([128, 128], BF16, tag="cs", name=f"cs{c}_{qb}")
                    nc.vector.tensor_copy(cs[:], tp[:])
                    qtr = 2 * b + k // 2
                    m = 4 * (k % 2) + qb
                    nc.sync.dma_start(a2a_in[qtr][m, :, :], cs[:])

            def collective(qtr):
                nc.gpsimd.collective_compute(
                    "AllToAll", mybir.AluOpType.bypass,
                    replica_groups=[list(range(NCORES))],
                    ins=[a2a_in[qtr].opt()], outs=[a2a_out[qtr].opt()])

            def phase2_half(qtr, nh):
                key = ("ctxg", qtr)
                if key not in es:
                    g = ctxgp.tile([128, KO, RH], BF16, tag="ctxg",
                                   name=f"ctxg{qtr}")
                    nc.sync.dma_start(g[:], a2a_out[qtr][:].rearrange(
                        "j q r -> q j r"))
                    es[key] = g
                ctxg = es[key]
                wo_t = wo_holder["wo"]
                pj = ps.tile([128, 512], F32, tag="big", name=f"p2_{qtr}_{nh}")
                for j in range(KO):
                    nc.tensor.matmul(
                        pj[:], ctxg[:, j, :], wo_t[:, j, nh * 512:(nh + 1) * 512],
                        start=(j == 0), stop=(j == KO - 1))
                ob = osbp.tile([128, 512], F32, tag="osb", name=f"ob{qtr}_{nh}")
                nc.vector.tensor_copy(ob[:], pj[:])
                nc.sync.dma_start(
                    out.ap()[qtr * RH:(qtr + 1) * RH, nh * 512:(nh + 1) * 512],
                    ob[:])

            # ---------------- emission schedule ----------------

            def interleave(fg, bg):
                """emit fg items with bg items spread evenly between them."""
                nf, nb = len(fg), len(bg)
                bi = 0
                for i, f in enumerate(fg):
                    f()
                    want = (i + 1) * nb // nf
                    while bi < want:
                        bg[bi]()
                        bi += 1
                while bi < nb:
                    bg[bi]()
                    bi += 1

            def scores_items(c):
                return [(lambda h=h, kbp=kbp: score_pair(c, h, kbp))
                        for kbp in range(KBP) for h in range(2)]

            def ctx_items(c):
                return [(lambda h=h, qb=qb: ctx_group(c, h, qb))
                        for qb in range(NQB) for h in range(2)]

            def proj_items(b):
                its = []
                for rb in range(NCH):
                    its += [lambda b=b, rb=rb: proj_q(b, rb),
                            lambda b=b, rb=rb: proj_k(b, rb),
                            lambda b=b, rb=rb: proj_v(b, rb)]
                return its

            def dma_wo():
                wo_t = cpool.tile([128, KO, D], BF16, tag="wo", name="wo_t")
                nc.sync.dma_start(wo_t[:], wo[:])
                wo_holder["wo"] = wo_t

            def region(c_scores, ctx_c=None, extra=(), extra_front=()):
                """scores of chunk c_scores; ctx of ctx_c front-loaded into
                the first half, other work into the second half."""
                s = scores_items(c_scores)
                front = (list(extra_front) +
                         (ctx_items(ctx_c) if ctx_c is not None else []))
                interleave(s[0:8], front)
                interleave(s[8:16], list(extra))

            # prologue: project b0 blocks 0,1 so scores of chunk 0 can start
            pb0 = proj_items(0)
            for it in pb0[0:6]:
                it()
            # region 0: scores chunk 0; proj b0 blocks 2,3 (kt before kbp=2rb)
            s0 = scores_items(0)
            interleave(s0[0:8], pb0[6:12])
            for it in s0[8:16]:
                it()
            dma_wo()
            region(1, ctx_c=0, extra=proj_items(1)[0:6])
            region(2, ctx_c=1, extra=proj_items(1)[6:12])
            collective(0)
            region(3, ctx_c=2)
            region(4, ctx_c=3)
            collective(1)
            region(5, ctx_c=4)
            region(6, ctx_c=5)
            collective(2)
            region(7, ctx_c=6)
            # epilogue: ship the last chunk first so A2A3 triggers ASAP,
            # then fill its in-flight window with ALL the output projections
            # (quarters 0-2 have their data; quarter 3 lands last)
            for it in ctx_items(7):
                it()
            for q in range(3):
                phase2_half(q, 0)
                phase2_half(q, 1)
            collective(3)
            phase2_half(3, 0)
            phase2_half(3, 1)
    nc.compile()
    return nc


def _numpy_reference(tensor_in, attention_mask, Wq, Wk, Wv, Wo):
    """Fallback for a non-zero mask (never hit with the spec's zero mask)."""
    x = tensor_in.astype(np.float64)
    q = (x @ Wq.T.astype(np.float64)).reshape(B, S, H, HD).transpose(0, 2, 1, 3)
    k = (x @ Wk.T.astype(np.float64)).reshape(B, S, H, HD).transpose(0, 2, 1, 3)
    v = (x @ Wv.T.astype(np.float64)).reshape(B, S, H, HD).transpose(0, 2, 1, 3)
    scores = np.einsum("bhqd,bhkd->bhqk", q, k) + attention_mask.astype(np.float64)
    scores = scores / np.sqrt(D)
    scores -= scores.max(axis=-1, keepdims=True)
    w = np.exp(scores)
    w /= w.sum(axis=-1, keepdims=True)
    ctx = np.einsum("bhqk,bhkd->bhqd", w, v).transpose(0, 2, 1, 3).reshape(B, S, D)
    return (ctx @ Wo.T.astype(np.float64)).astype(np.float32)


def _pretile(wT: np.ndarray) -> np.ndarray:
    """[D, M] -> [128, KO, M] with row d = ko*128 + p."""
    m = wT.shape[1]
    return np.ascontiguousarray(wT.reshape(KO, 128, m).transpose(1, 0, 2))


def _row_map() -> np.ndarray:
    """global row index handled by (core c, local row lr)."""
    m = np.empty((NCORES, RPC), dtype=np.int64)
    for c in range(NCORES):
        for qtr in range(NQTR):
            b, j = qtr // 2, qtr % 2
            g = b * S + j * 1024 + c * 128
            m[c, qtr * 128:(qtr + 1) * 128] = np.arange(g, g + 128)
    return m


def make_in_maps(inputs):
    import ml_dtypes

    bf16 = ml_dtypes.bfloat16
    f8 = ml_dtypes.float8_e4m3fn
    tensor_in = np.asarray(inputs["tensor_in"], dtype=np.float32)
    Wq = np.asarray(inputs["Wq"], dtype=np.float32)
    Wk = np.asarray(inputs["Wk"], dtype=np.float32)
    Wv = np.asarray(inputs["Wv"], dtype=np.float32)
    Wo = np.asarray(inputs["Wo"], dtype=np.float32)

    xTf = np.ascontiguousarray(tensor_in.reshape(ROWS, D).T)
    xT = xTf.astype(bf16)
    xT8 = xTf.astype(f8)
    wqT = (Wq.T * WS).astype(f8)
    wkT = (Wk.T * WS).astype(f8)
    wvT = Wv.T.astype(bf16)
    wo_p = _pretile(Wo.T.astype(bf16))

    in_maps = []
    for c in range(NCORES):
        sl = slice(c * CH, (c + 1) * CH)
        in_maps.append({
            "xT": xT,
            "xT8": xT8,
            "wq": _pretile(wqT[:, sl]),
            "wk": _pretile(wkT[:, sl]),
            "wv": _pretile(wvT[:, sl]),
            "wo": wo_p,
        })
    return in_maps


def _run(inputs, trace=False):
    from concourse.bass_utils import run_bass_kernel_spmd

    in_maps = make_in_maps(inputs)
    if "nc" not in _CACHE:
        _CACHE["nc"] = _build()
    res = run_bass_kernel_spmd(
        _CACHE["nc"], in_maps, core_ids=list(range(NCORES)), trace=trace
    )
    rm = _CACHE.setdefault("rm", _row_map())
    full = np.empty((ROWS, D), dtype=np.float32)
    for c in range(NCORES):
        full[rm[c]] = res.results[c]["out"]
    return full.reshape(B, S, D), res


def kernel(**inputs) -> np.ndarray:
    mask = np.asarray(inputs["attention_mask"])
    if mask.any():
        return _numpy_reference(
            np.asarray(inputs["tensor_in"]), mask,
            np.asarray(inputs["Wq"]), np.asarray(inputs["Wk"]),
            np.asarray(inputs["Wv"]), np.asarray(inputs["Wo"]),
        )
    out, _ = _run(inputs, trace=False)
    return out
